# revision 37
# baseline (speedup 1.0000x reference)
"""Multi-head attention (B=4, S=2048, D=1024, H=16) on 8 Trainium2 NeuronCores.

Sharding: core c = (batch b = c//2, head-group hg = c%2). Each core computes
heads hg*8..hg*8+7 for batch b over the full sequence, producing a partial
output o_c[s, :] = ctx_c @ Wo[:, hg-dims].T (+ bo on hg==0 cores). The host
sums the two partial outputs per batch. Exact decomposition: each core does
1/8 of the FLOPs with no cross-core communication.

Per-core dataflow (matmul inputs bf16, accumulation fp32), ACT-paced design:
  The kernel is organized so the ScalarE (ACT) engine -- whose 256 exp
  instructions are the irreducible floor -- never waits:
  - blocks = (head-pair lj, 512-query pass qp); 16 k-iterations per block.
  - per iteration: ONE co-issued score matmul pair (head0 on PE rows 0:64,
    head1 on rows 64:128 -- the PE runs both concurrently via row-group
    tiling), then ONE [128,1024] exp covering both heads, then the PV pair
    for iteration k-2 (2-deep software pipeline).
  - s01 score PSUM tiles are double-buffered, so exp(g) depends on scores
    emitted 2 iterations earlier: the ACT queue streams back-to-back.
  - denominators ride along as a 65th ones-column in V'; normalization is
    entirely off the ACT path: DVE reciprocal (cross-partition), GpSimd
    partition_broadcast, DVE multiply (fused with the ctx copy) + bias add.
  - K/Q projections for pair lj+1, the V projection, and the output
    projection are interleaved into the k-iterations at a rate that fits
    under the exp cadence.
"""

from contextlib import ExitStack

import ml_dtypes
import numpy as np

import concourse.bass as bass
import concourse.tile as tile
from concourse import bacc, mybir
from concourse.bass_utils import run_bass_kernel_spmd

BF16 = mybir.dt.bfloat16
F32 = mybir.dt.float32
NPBF16 = ml_dtypes.bfloat16

B, S, D, H, DK = 4, 2048, 1024, 16, 64
N_CORES = 8
HG = H // 2  # heads per core
NPAIR = HG // 2  # head pairs per core
ND = D // 128  # contraction d-tiles
NT = S // 128  # token tiles (k-iterations per block)
NQP = 4  # query passes of 512
QW = S // NQP  # query window (512)
DH = HG * DK  # 512: output dims per core
E = DK + 1  # V' columns per head (64 + ones)
SCALE = 1.0 / np.sqrt(DK)
EXP = mybir.ActivationFunctionType.Exp


def _emit(tc, tin, tout):
    nc = tc.nc
    with ExitStack() as ctx:
        SP = ctx.enter_context(tc.tile_pool(name="static", bufs=1))
        PS = ctx.enter_context(tc.tile_pool(name="psum", bufs=2, space="PSUM"))
        KTP = ctx.enter_context(tc.tile_pool(name="ktp", bufs=2))
        QTP = ctx.enter_context(tc.tile_pool(name="qtp", bufs=2))
        WKP = ctx.enter_context(tc.tile_pool(name="wkp", bufs=2))
        WQP = ctx.enter_context(tc.tile_pool(name="wqp", bufs=2))
        P01 = ctx.enter_context(tc.tile_pool(name="p01", bufs=5))
        BCF = ctx.enter_context(tc.tile_pool(name="bcf", bufs=4))
        DEN = ctx.enter_context(tc.tile_pool(name="denp", bufs=4))
        OSP = ctx.enter_context(tc.tile_pool(name="osp", bufs=3))

        # ---- constants ----
        bq_all = SP.tile([128, NPAIR], F32, tag="bq_all")
        nc.sync.dma_start(bq_all[:], tin["bqc"][:, :])
        bv_all = SP.tile([128, NPAIR], F32, tag="bv_all")
        nc.sync.dma_start(bv_all[:], tin["bvc"][:, :])
        zexp = SP.tile([128, 1], F32, tag="zexp")
        nc.vector.memset(zexp[:], 0.0)
        ones64 = SP.tile([128, DK], BF16, tag="ones64")
        nc.vector.memset(ones64[:], 1.0)

        # ---- static loads ----
        dmae = [nc.sync, nc.gpsimd]
        zts = [SP.tile([128, S], BF16, tag=f"zt{d}", name=f"zt{d}") for d in range(ND)]
        wvs = [
            SP.tile([128, DH], BF16, tag=f"wv{d}", name=f"wv{d}") for d in range(ND)
        ]

        def load_z_quarter(quarter):
            csl = slice(quarter * 512, (quarter + 1) * 512)
            for d in range(ND):
                dmae[d % 2].dma_start(
                    zts[d][:, csl], tin["ztc"][d * 128 : (d + 1) * 128, csl]
                )

        def load_wv():
            for d in range(ND):
                dmae[d % 2].dma_start(
                    wvs[d][:], tin["wvTc"][d * 128 : (d + 1) * 128, :]
                )

        # V' tiles: [128 tokens, 8 heads x (64 dims + ones col)]
        vsb = []
        for t in range(NT):
            v_ = SP.tile([128, HG * E], BF16, tag=f"vsb{t}", name=f"vsb{t}")
            nc.vector.memset(
                v_.rearrange("p (h e) -> p h e", e=E)[:, :, DK : DK + 1], 1.0
            )
            vsb.append(v_)

        # normalized context, bf16: per pair [128 dims, S queries]
        ctxu = []
        for lj in range(NPAIR):
            cu = SP.tile([128, S], BF16, tag=f"ctxu{lj}", name=f"ctxu{lj}")
            ctxu.append(cu)

        def emit_vproj(t):
            ps = PS.tile([128, DH], F32, tag="proj", name=f"psv{t}")
            for d in range(ND):
                nc.tensor.matmul(
                    ps[:],
                    lhsT=zts[d][:, t * 128 : (t + 1) * 128],
                    rhs=wvs[d][:],
                    start=(d == 0),
                    stop=(d == ND - 1),
                )
            nc.vector.tensor_copy(
                vsb[t].rearrange("p (h e) -> p h e", e=E)[:, :, 0:DK],
                ps.rearrange("p (h e) -> p h e", e=DK),
            )

        def emit_proj_dmas(lj):
            jsl = slice(lj * 128, (lj + 1) * 128)
            wkj = WKP.tile([128, ND * 128], BF16, tag="wk", name=f"wk_{lj}")
            nc.sync.dma_start(
                wkj.rearrange("p (d j) -> p d j", j=128),
                tin["wkTc"][:, jsl].rearrange("(d p) j -> p d j", p=128),
            )
            wqj = WQP.tile([128, ND * 128], BF16, tag="wq", name=f"wq_{lj}")
            nc.gpsimd.dma_start(
                wqj.rearrange("p (d j) -> p d j", j=128),
                tin["wqTc"][:, jsl].rearrange("(d p) j -> p d j", p=128),
            )
            kt = KTP.tile([128, S], BF16, tag="kt", name=f"kt{lj}")
            qt = QTP.tile([128, S], BF16, tag="qt", name=f"qt{lj}")
            return (lj, wkj, wqj, kt, qt)

        proj_ps = {}

        def emit_proj_part(pst, i, dp):
            """Quarter of a K/Q-projection psum group: 2 of the 8 d-matmuls
            (dp in 0..3), evac on the last part. Spreading parts across
            k-iterations keeps per-iteration PE work under the exp cadence.
            i in 0..3 -> K chunk i (tokens 512i..512i+512); 4..7 -> Q chunk."""
            lj, wkj, wqj, kt, qt = pst
            tcx = i % 4
            sl = slice(tcx * 512, (tcx + 1) * 512)
            w, pref = (wkj, "psk") if i < 4 else (wqj, "psq")
            key = (lj, i)
            if dp == 0:
                proj_ps[key] = PS.tile(
                    [128, 512], F32, tag="proj", name=f"{pref}{lj}_{tcx}"
                )
            ps = proj_ps[key]
            for d in (2 * dp, 2 * dp + 1):
                nc.tensor.matmul(
                    ps[:],
                    lhsT=w[:, d * 128 : (d + 1) * 128],
                    rhs=zts[d][:, sl],
                    start=(d == 0),
                    stop=(d == ND - 1),
                )
            if dp == 3:
                if i < 4:
                    nc.vector.tensor_copy(kt[:, sl], ps[:])
                else:
                    nc.vector.tensor_scalar_add(
                        qt[:, sl], ps[:], bq_all[:, lj : lj + 1]
                    )
                del proj_ps[key]

        def emit_proj_chunk(pst, i):
            for dp in range(4):
                emit_proj_part(pst, i, dp)

        # ---- output projection ----
        wos = []
        bo_sb = None

        def load_phase3_weights():
            nonlocal bo_sb
            for pl in range(NPAIR):
                wo_ = SP.tile([128, D], BF16, tag=f"wo{pl}", name=f"wo{pl}")
                nc.sync.dma_start(wo_[:], tin["woTc"][pl * 128 : (pl + 1) * 128, :])
                wos.append(wo_)
            bo_sb = SP.tile([128, D], F32, tag="bo_sb")
            boap = tin["boc"]
            nc.gpsimd.dma_start(
                bo_sb[:],
                bass.AP(
                    tensor=boap.tensor, offset=boap.offset, ap=[[0, 128], [1, D]]
                ),
            )

        ost_tiles = {}
        p3_ps = {}

        def emit_phase3_half(st, jc, half, tag="proj"):
            """Half of an output chunk: 2 of the 4 pair-contraction matmuls
            for o[st*128:(st+1)*128, jc*512:...]; bias-add + DMA on the last."""
            ssl = slice(st * 128, (st + 1) * 128)
            jsl = slice(jc * 512, (jc + 1) * 512)
            if st not in ost_tiles:
                ost_tiles[st] = OSP.tile([128, D], F32, tag="ost", name=f"ost{st}")
            ost = ost_tiles[st]
            key = (st, jc)
            if half == 0:
                p3_ps[key] = PS.tile(
                    [128, 512], F32, tag=tag, name=f"pso{st}_{jc}"
                )
            ps = p3_ps[key]
            for l in (2 * half, 2 * half + 1):
                nc.tensor.matmul(
                    ps[:], lhsT=ctxu[l][:, ssl], rhs=wos[l][:, jsl],
                    start=(l == 0), stop=(l == NPAIR - 1),
                )
            if half == 1:
                nc.vector.tensor_add(ost[:, jsl], ps[:], bo_sb[:, jsl])
                del p3_ps[key]
                if jc == 1:
                    nc.sync.dma_start(tout["o"][ssl, :], ost[:])

        # ---- finalize one block: normalize both heads off the ACT path.
        # Part a (end of block): evacuate both ctx psum tiles (values fp32 +
        # denominator row bf16) -- after these four DVE copies the ctx psum
        # slots are free, so the next block's PV never waits on the rest of
        # the normalize chain. Part b (next block, k==0): broadcast the
        # denominators with a co-issued col-tiled K=1 matmul pair,
        # reciprocal, multiply, bias -- all SBUF-side.
        def emit_finalize_a(lj, qp, ctx0, ctx1):
            den0 = DEN.tile([65, QW], BF16, tag="den", name=f"den0_{lj}_{qp}")
            nc.vector.tensor_copy(den0[64:65, :], ctx0[64:65, :])
            den1 = DEN.tile([65, QW], BF16, tag="den", name=f"den1_{lj}_{qp}")
            nc.vector.tensor_copy(den1[64:65, :], ctx1[64:65, :])
            return (lj, qp, ctx0, ctx1, den0, den1)

        def emit_finalize_b(lj, qp, ctx0, ctx1, den0, den1):
            qsl = slice(qp * QW, (qp + 1) * QW)
            bc = PS.tile([128, QW], F32, tag="proj", name=f"bc_{lj}_{qp}")
            nc.tensor.matmul(
                bc[0:64, :], lhsT=ones64[64:65, 0:64], rhs=den0[64:65, :],
                start=True, stop=True,
            )
            nc.tensor.matmul(
                bc[64:128, :], lhsT=ones64[64:65, 0:64], rhs=den1[64:65, :],
                start=True, stop=True,
            )
            bcr = BCF.tile([128, QW], F32, tag="bcf", name=f"bcr_{lj}_{qp}")
            nc.vector.reciprocal_approx_fast(out=bcr[:], in_=bc[:])
            # normalize fused with the ctx copy: ctxu = ctx * (1/den)
            nc.vector.tensor_mul(ctxu[lj][0:64, qsl], ctx0[0:64, :], bcr[0:64, :])
            nc.vector.tensor_mul(ctxu[lj][64:128, qsl], ctx1[0:64, :], bcr[64:128, :])
            # bv is exact post-normalization (sum_k p = den)
            nc.vector.tensor_scalar_add(
                ctxu[lj][:, qsl], ctxu[lj][:, qsl], bv_all[:, lj : lj + 1]
            )

        # ================= schedule =================
        # startup: pair-0 weights + z d-tiles first (8 big DMAs), Wv behind;
        # then K0/Q0 chunks and the first two V' tiles gate block (0,0).
        proj_cur = emit_proj_dmas(0)
        load_z_quarter(0)
        load_wv()
        load_z_quarter(1)
        # warm the PE HAM clock-gate (idle default is 1.2 GHz; ~3.4us of
        # sustained matmul activity unlocks 2.4 GHz) during the z DMA wait
        warm = SP.tile([128, 512], BF16, tag="warm")
        nc.vector.memset(warm[:], 0.01)
        pw = PS.tile([128, 512], F32, tag="proj", name="pwarm")
        for i in range(20):
            nc.tensor.matmul(
                pw[:], lhsT=warm[:, 0:128], rhs=warm[:],
                start=(i == 0), stop=(i == 19),
            )
        emit_proj_chunk(proj_cur, 0)  # K tokens 0..511
        emit_proj_chunk(proj_cur, 4)  # Q tokens 0..511
        load_z_quarter(2)
        load_z_quarter(3)

        # per-block interleave schedules
        # pair lj's blocks carry: remaining K chunks (iters 2,6 of qp0),
        # Q chunk qp+1 (iter 10 of qp), next pair's DMAs + chunks in qp2/qp3,
        # vproj 2..15 in block (0,0), phase3 for pair-3 passes.
        blocks = [(lj, qp) for lj in range(NPAIR) for qp in range(NQP)]
        kt_cur, qt_cur = proj_cur[3], proj_cur[4]
        kt_next = qt_next = None
        proj_next = None
        pvq = []  # deferred PV work: (k, p01, ctx0, ctx1, vs0, vs1)
        fin_pend = None

        for bi, (lj, qp) in enumerate(blocks):
            q0 = qp * QW
            ctx0 = PS.tile([65, QW], F32, tag="ctx", name=f"ctx0_{lj}_{qp}")
            ctx1 = PS.tile([65, QW], F32, tag="ctx", name=f"ctx1_{lj}_{qp}")
            for k in range(NT):
                ksl = slice(k * 128, (k + 1) * 128)
                qsl = slice(q0, q0 + QW)
                # --- co-issued score pair: head0 rows 0:64, head1 rows 64:128
                s01 = PS.tile([128, 2 * QW], F32, tag="s01", name=f"s_{bi}_{k}")
                nc.tensor.matmul(
                    s01[:, 0:QW],
                    lhsT=kt_cur[0:64, ksl], rhs=qt_cur[0:64, qsl],
                    start=True, stop=True,
                )
                nc.tensor.matmul(
                    s01[:, QW : 2 * QW],
                    lhsT=kt_cur[64:128, ksl], rhs=qt_cur[64:128, qsl],
                    start=True, stop=True,
                )
                # --- one exp for both heads
                p01 = P01.tile([128, 2 * QW], BF16, tag="p01", name=f"p_{bi}_{k}")
                nc.scalar.activation(p01[:], s01[:], EXP, bias=zexp[:], scale=SCALE)
                pvq.append((k, p01))
                # --- second half of the previous block's finalize
                if fin_pend is not None and k == 0:
                    emit_finalize_b(*fin_pend)
                    fin_pend = None
                # --- deferred PV pair (k-2)
                if len(pvq) > 2:
                    kk, pp = pvq.pop(0)
                    h0 = 2 * lj
                    nc.tensor.matmul(
                        ctx0[:],
                        lhsT=vsb[kk][:, h0 * E : h0 * E + E],
                        rhs=pp[:, 0:QW],
                        start=(kk == 0), stop=(kk == NT - 1),
                    )
                    nc.tensor.matmul(
                        ctx1[:],
                        lhsT=vsb[kk][:, (h0 + 1) * E : (h0 + 1) * E + E],
                        rhs=pp[:, QW : 2 * QW],
                        start=(kk == 0), stop=(kk == NT - 1),
                    )
                # --- interleaved projection / vproj / phase-3 work at 2-MM
                # granularity, so per-iteration PE work stays under the exp
                # cadence (scores 216 + PV 432 + 2 proj MMs 432 < 1147ns).
                # Parts must precede first use in program order: the PE queue
                # is in-order, so a score matmul can never wait on a
                # projection emitted after it.
                if bi == 0:
                    # K chunk c gates score iter 4c; Q1 gates block (0,1)
                    if k in (1, 2):
                        emit_proj_part(proj_cur, 1, 2 * (k - 1))
                        emit_proj_part(proj_cur, 1, 2 * (k - 1) + 1)
                    elif 3 <= k <= 6:
                        emit_proj_part(proj_cur, 2, k - 3)
                    elif 7 <= k <= 10:
                        emit_proj_part(proj_cur, 3, k - 7)
                    elif 11 <= k <= 14:
                        emit_proj_part(proj_cur, 5, k - 11)
                    # the first two V' tiles ride inside iter 0, off the
                    # startup critical path: PV(0) only needs them by iter 2
                    if k == 0:
                        emit_vproj(0)
                        emit_vproj(1)
                    if k < NT - 2:
                        emit_vproj(k + 2)
                elif qp == 0 and bi > 0:
                    if 4 <= k <= 7:
                        emit_proj_part(proj_cur, 5, k - 4)  # Q chunk for qp1
                elif qp == 1:
                    if lj == 0 and k == 0:
                        load_phase3_weights()
                    if 4 <= k <= 7:
                        emit_proj_part(proj_cur, 6, k - 4)  # Q chunk for qp2
                    if lj + 1 < NPAIR:
                        if k == 3:
                            proj_next = emit_proj_dmas(lj + 1)
                            kt_next, qt_next = proj_next[3], proj_next[4]
                        elif 8 <= k <= 11:
                            emit_proj_part(proj_next, 0, k - 8)  # K0 next
                        elif 12 <= k <= 15:
                            emit_proj_part(proj_next, 1, k - 12)  # K1 next
                elif qp == 2:
                    if 2 <= k <= 5:
                        emit_proj_part(proj_cur, 7, k - 2)  # Q chunk for qp3
                    if lj + 1 < NPAIR:
                        if 6 <= k <= 9:
                            emit_proj_part(proj_next, 2, k - 6)  # K2 next
                        elif 10 <= k <= 13:
                            emit_proj_part(proj_next, 3, k - 10)  # K3 next
                        elif k >= 14:
                            emit_proj_part(proj_next, 4, k - 14)  # Q0 next 0,1
                elif qp == 3:
                    if lj + 1 < NPAIR and k <= 1:
                        emit_proj_part(proj_next, 4, 2 + k)  # Q0 next 2,3
                # phase 3: after pair-3 finishes qpass qp (finalized at the
                # start of the next block), s-tiles 4qp..4qp+3 are ready;
                # 16 half-chunks (2 MMs each) spread over k=1..15.
                if lj == NPAIR - 1 and qp > 0 and k >= 1:
                    done_qp = qp - 1
                    hs = [k - 1] if k < 15 else [14, 15]
                    for h in hs:
                        emit_phase3_half(
                            done_qp * 4 + h // 4, (h % 4) // 2, h % 2
                        )
            # Q chunk 0 of qpass 0 for first block already done in startup.
            # finalize deferred to next block's k=0 (after PV drains below)
            while pvq:
                kk, pp = pvq.pop(0)
                h0 = 2 * lj
                nc.tensor.matmul(
                    ctx0[:],
                    lhsT=vsb[kk][:, h0 * E : h0 * E + E],
                    rhs=pp[:, 0:QW],
                    start=(kk == 0), stop=(kk == NT - 1),
                )
                nc.tensor.matmul(
                    ctx1[:],
                    lhsT=vsb[kk][:, (h0 + 1) * E : (h0 + 1) * E + E],
                    rhs=pp[:, QW : 2 * QW],
                    start=(kk == 0), stop=(kk == NT - 1),
                )
            fin_pend = emit_finalize_a(lj, qp, ctx0, ctx1)
            if qp == NQP - 1 and lj + 1 < NPAIR:
                kt_cur, qt_cur = kt_next, qt_next
                proj_cur = proj_next

        # ---- tail: finalize the last block, then the last 4 s-tiles.
        # pso tiles alternate between the (now free) s01 slots and the proj
        # ring so 4 psum groups are in flight and the PE streams the 32
        # matmuls back-to-back while the DVE bias-adds trail.
        emit_finalize_b(*fin_pend)
        for st in range(4 * (NQP - 1), NT):
            tag = "s01" if st % 2 == 0 else "proj"
            for jc in range(2):
                emit_phase3_half(st, jc, 0, tag=tag)
                emit_phase3_half(st, jc, 1, tag=tag)


def build_nc():
    nc = bacc.Bacc(
        "TRN2", target_bir_lowering=False, debug=False, num_devices=N_CORES
    )
    tin = {
        "ztc": nc.dram_tensor("ztc", [D, S], BF16, kind="ExternalInput").ap(),
        "wqTc": nc.dram_tensor("wqTc", [D, DH], BF16, kind="ExternalInput").ap(),
        "wkTc": nc.dram_tensor("wkTc", [D, DH], BF16, kind="ExternalInput").ap(),
        "wvTc": nc.dram_tensor("wvTc", [D, DH], BF16, kind="ExternalInput").ap(),
        "woTc": nc.dram_tensor("woTc", [DH, D], BF16, kind="ExternalInput").ap(),
        "bqc": nc.dram_tensor("bqc", [128, NPAIR], F32, kind="ExternalInput").ap(),
        "bvc": nc.dram_tensor("bvc", [128, NPAIR], F32, kind="ExternalInput").ap(),
        "boc": nc.dram_tensor("boc", [1, D], F32, kind="ExternalInput").ap(),
    }
    tout = {"o": nc.dram_tensor("o", [S, D], F32, kind="ExternalOutput").ap()}
    with tile.TileContext(nc) as tc:
        _emit(tc, tin, tout)
    nc.compile()
    return nc


_NC = None


def _get_nc():
    global _NC
    if _NC is None:
        _NC = build_nc()
    return _NC


def make_in_maps(z, Wq, bq, Wk, Wv, bv, Wo, bo):
    """Build the 8 per-core input maps from full fp32 inputs."""
    z = np.asarray(z, np.float32)
    bq = np.asarray(bq, np.float32)
    bv = np.asarray(bv, np.float32)
    bo = np.asarray(bo, np.float32)
    wqT = np.asarray(Wq, np.float32).T
    wkT = np.asarray(Wk, np.float32).T
    wvT = np.asarray(Wv, np.float32).T
    woT = np.asarray(Wo, np.float32).T
    zts = [np.ascontiguousarray(z[b].T).astype(NPBF16) for b in range(B)]
    per_hg = []
    for hg in range(2):
        dsl = slice(hg * DH, (hg + 1) * DH)
        per_hg.append(
            {
                "wqTc": np.ascontiguousarray(wqT[:, dsl]).astype(NPBF16),
                "wkTc": np.ascontiguousarray(wkT[:, dsl]).astype(NPBF16),
                "wvTc": np.ascontiguousarray(wvT[:, dsl]).astype(NPBF16),
                "woTc": np.ascontiguousarray(woT[dsl, :]).astype(NPBF16),
                "bqc": np.ascontiguousarray(bq[dsl].reshape(NPAIR, 128).T),
                "bvc": np.ascontiguousarray(bv[dsl].reshape(NPAIR, 128).T),
                "boc": bo.reshape(1, D) if hg == 0 else np.zeros((1, D), np.float32),
            }
        )
    in_maps = []
    for c in range(N_CORES):
        b, hg = c // 2, c % 2
        in_maps.append({"ztc": zts[b], **per_hg[hg]})
    return in_maps


def run(in_maps, trace=False):
    nc = _get_nc()
    return run_bass_kernel_spmd(
        nc, in_maps, core_ids=list(range(N_CORES)), trace=trace
    )


def kernel(z, Wq, bq, Wk, bk, Wv, bv, Wo, bo):
    in_maps = make_in_maps(z, Wq, bq, Wk, Wv, bv, Wo, bo)
    res = run(in_maps)
    out = np.empty((B, S, D), np.float32)
    for b in range(B):
        out[b] = res.results[2 * b]["o"] + res.results[2 * b + 1]["o"]
    return out
